# revision 1
# baseline (speedup 1.0000x reference)
"""Fused GPT-2 transformer block on 8 Trainium2 NeuronCores.

Sharding: 8 cores = 4 batches x 2 causal-balanced folds. Core (b, f) owns the 8
interleaved 128-token blocks of parity f of batch b (queries), and receives all
2048 tokens of batch b as context, permuted [other-parity blocks | own blocks].
Causality is enforced by a per-core additive mask shipped as data, so a single
SPMD program serves all cores. No collectives.

Layouts: LN1(x) is PE-transposed to hT [D, tok]; Q/K are produced directly in
head-major transposed layout, V in token-major layout with an appended ones
column (so the P@V matmul also accumulates softmax denominators). Attention
runs fully in the transposed layout; proj/fc2 contract against feature-major
lhsT slices, landing outputs back in token-major layout for residuals/LN.
All matmuls run in float32r (full PE rate, ~1.5e-4 rel err).
"""

import contextlib
import os

import numpy as np

import concourse.bass as bass
import concourse.mybir as mybir
import concourse.tile as tile
from concourse import bacc
from concourse.bass_utils import run_bass_kernel_spmd
from concourse.masks import make_identity

F32 = mybir.dt.float32
F32R = mybir.dt.float32r
AF = mybir.ActivationFunctionType
ALU = mybir.AluOpType

B, S, D, H = 4, 2048, 1024, 16
HD = D // H          # 64
DFF = 4 * D          # 4096
EPS = 1e-5
MASKED_BIAS = -10000.0
N_CORES = 8

SB = S // 128        # 16 ctx blocks
OWN = S // 2         # 1024 own tokens
OB = OWN // 128      # 8 own blocks
NQG = 4              # q-groups of 256
QG = 256
HSETS = 4            # head sets
HPS = H // HSETS     # 4 heads per set


def _klist(g):
    """ctx k-block indices computed for q-group g (own blocks 2g, 2g+1)."""
    return list(range(0, 2 * g + 2)) + list(range(8, 8 + 2 * g + 2))


def build_nc(am_zero=True):
    nc = bacc.Bacc("TRN2", target_bir_lowering=False, debug=False,
                   num_devices=N_CORES)

    X = nc.dram_tensor("X", [S, D], F32, kind="ExternalInput")
    MSK = (nc.dram_tensor("MSK", [2, 128, 512], F32, kind="ExternalInput")
           if am_zero else
           nc.dram_tensor("MSK", [16, 128, QG], F32, kind="ExternalInput"))
    AM = nc.dram_tensor("AM", [128, SB], F32, kind="ExternalInput")
    WQ = nc.dram_tensor("WQ", [D, D], F32, kind="ExternalInput")
    WK = nc.dram_tensor("WK", [D, D], F32, kind="ExternalInput")
    WV = nc.dram_tensor("WV", [D, D], F32, kind="ExternalInput")
    BQ = nc.dram_tensor("BQ", [D, 1], F32, kind="ExternalInput")
    BK = nc.dram_tensor("BK", [D, 1], F32, kind="ExternalInput")
    BV = nc.dram_tensor("BV", [1, D], F32, kind="ExternalInput")
    WP = nc.dram_tensor("WP", [D, D], F32, kind="ExternalInput")
    BP = nc.dram_tensor("BP", [1, D], F32, kind="ExternalInput")
    WF = nc.dram_tensor("WF", [D, DFF], F32, kind="ExternalInput")
    BF = nc.dram_tensor("BF", [DFF, 1], F32, kind="ExternalInput")
    WF2 = nc.dram_tensor("WF2", [DFF, D], F32, kind="ExternalInput")
    BF2 = nc.dram_tensor("BF2", [1, D], F32, kind="ExternalInput")
    OUT = nc.dram_tensor("OUT", [OWN, D], F32, kind="ExternalOutput")

    with tile.TileContext(nc) as tc:
        _body(nc, tc, X, MSK, AM, WQ, WK, WV, BQ, BK, BV, WP, BP, WF, BF,
              WF2, BF2, OUT, am_zero)
    nc.compile()
    return nc


def _layernorm_tile(nc, stat, src_tile, eps_t):
    """In-place LN (no affine) of src_tile [128, D]."""
    sub = 512
    nsub = D // sub
    xs = src_tile[:, :].rearrange("p (n s) -> p n s", s=sub)
    stats = stat.tile([128, nsub, nc.vector.BN_STATS_DIM], F32, tag="bnst")
    for j in range(nsub):
        nc.vector.bn_stats(out=stats[:, j, :], in_=xs[:, j, :])
    mv = stat.tile([128, nc.vector.BN_AGGR_DIM], F32, tag="bnag")
    nc.vector.bn_aggr(out=mv[:, :], in_=stats[:, :, :])
    nc.scalar.activation(out=mv[:, 1:2], in_=mv[:, 1:2], func=AF.Sqrt,
                         bias=eps_t[:], scale=1.0)
    nc.vector.reciprocal(out=mv[:, 1:2], in_=mv[:, 1:2])
    nc.vector.tensor_scalar(out=src_tile[:, :], in0=src_tile[:, :],
                            scalar1=mv[:, 0:1], scalar2=mv[:, 1:2],
                            op0=ALU.subtract, op1=ALU.mult)


def _body(nc, tc, X, MSK, AM, WQ, WK, WV, BQ, BK, BV, WP, BP, WF, BF, WF2,
          BF2, OUT, am_zero=True):
    PL = int(os.environ.get("KPHASES", "9"))
    CH = 8  # exp-staging chunk, in k-tiles
    with contextlib.ExitStack() as top:
        cst = top.enter_context(tc.tile_pool(name="cst", bufs=1))
        stat = top.enter_context(tc.tile_pool(name="stat", bufs=4))

        ident = cst.tile([128, 128], F32)
        make_identity(nc, ident[:])
        ones_f = cst.tile([1, 128], F32)
        nc.vector.memset(ones_f[:], 1.0)
        ones_c3 = cst.tile([128, HPS, 1], F32)
        nc.vector.memset(ones_c3[:], 1.0)
        ones_r = cst.tile([1, 128], F32R)   # bias-row lhsT
        nc.scalar.copy(ones_r[:], ones_f[:])
        ones_b = cst.tile([1, 64], F32R)    # denominator-broadcast lhsT
        nc.scalar.copy(ones_b[:], ones_f[:, 0:64])
        eps_t = cst.tile([128, 1], F32)
        nc.vector.memset(eps_t[:], EPS)

        with contextlib.ExitStack() as attn_stack:
            atp = attn_stack.enter_context(tc.tile_pool(name="atp", bufs=1))
            aT = [atp.tile([128, OWN], F32R, tag=f"aT{p}", name=f"aT{p}")
                  for p in range(8)]

            with contextlib.ExitStack() as ht_stack:
                ht = ht_stack.enter_context(tc.tile_pool(name="ht", bufs=1))

                # ---- Phase 1: LN1 over ctx + transpose -> hT ----
                hT = [ht.tile([128, S], F32R, tag=f"hT{db}", name=f"hT{db}")
                      for db in range(8)]
                with tc.tile_pool(name="psT", bufs=6, space="PSUM") as psT, \
                     tc.tile_pool(name="xin1", bufs=5) as xin:
                    for tb in range(SB):
                        x_t = xin.tile([128, D], F32, tag="x1")
                        nc.sync.dma_start(x_t[:], X[tb * 128:(tb + 1) * 128, :])
                        _layernorm_tile(nc, stat, x_t, eps_t)
                        for db in range(8):
                            pt = psT.tile([128, 128], F32, tag="tp")
                            nc.tensor.transpose(
                                pt[:], x_t[:, db * 128:(db + 1) * 128],
                                ident[:])
                            nc.vector.tensor_copy(
                                hT[db][:, tb * 128:(tb + 1) * 128], pt[:])

                if PL < 2:
                    return
                with contextlib.ExitStack() as hs_stack:
                    kvq = hs_stack.enter_context(
                        tc.tile_pool(name="kvq", bufs=1))
                    mskp = hs_stack.enter_context(
                        tc.tile_pool(name="mskp", bufs=1))
                    att = hs_stack.enter_context(
                        tc.tile_pool(name="att", bufs=3))
                    wst = hs_stack.enter_context(
                        tc.tile_pool(name="wstA", bufs=1))
                    psKV = hs_stack.enter_context(
                        tc.tile_pool(name="psKV", bufs=2, space="PSUM"))
                    psS = hs_stack.enter_context(
                        tc.tile_pool(name="psS", bufs=2, space="PSUM"))
                    psA = hs_stack.enter_context(
                        tc.tile_pool(name="psA", bufs=2, space="PSUM"))

                    # masks + attention-mask bias, loaded once
                    if am_zero:
                        mskB = mskp.tile([128, 512], F32, tag="mskB",
                                         name="mskB")
                        nc.sync.dma_start(mskB[:], MSK[0, :, :])
                        mskD = mskp.tile([128, 512], F32, tag="mskD",
                                         name="mskD")
                        nc.sync.dma_start(mskD[:], MSK[1, :, :])
                        msk_t = None
                    else:
                        msk_t = []
                        for m in range(16):
                            m_t = mskp.tile([128, QG], F32, tag=f"msk{m}",
                                            name=f"msk{m}")
                            nc.sync.dma_start(m_t[:], MSK[m, :, :])
                            msk_t.append(m_t)
                    am_sb = mskp.tile([128, SB], F32, tag="am", name="am")
                    nc.sync.dma_start(am_sb[:], AM[:, :])

                    for hs in range(HSETS):
                        # ---- Phase 2: K/V/Q projections for this head set ----
                        kT = [kvq.tile([128, S], F32R, tag=f"kT{p}",
                                       name=f"kT{p}") for p in range(2)]
                        qT = [kvq.tile([128, OWN], F32R, tag=f"qT{p}",
                                       name=f"qT{p}") for p in range(2)]
                        vS = [kvq.tile([128, HPS, HD + 1], F32R,
                                       tag=f"vS{tb}", name=f"vS{tb}")
                              for tb in range(SB)]

                        for p in range(2):
                            fcol = hs * 256 + p * 128
                            bq_c = stat.tile([128, 1], F32, tag="bqc")
                            nc.sync.dma_start(bq_c[:], BQ[fcol:fcol + 128, :])
                            bk_c = stat.tile([128, 1], F32, tag="bkc")
                            nc.sync.dma_start(bk_c[:], BK[fcol:fcol + 128, :])
                            wkt, wqt = [], []
                            for db in range(8):
                                w_t = wst.tile([128, 128], F32R,
                                               tag=f"wk{db}", name=f"wk{db}")
                                nc.sync.dma_start(
                                    w_t[:],
                                    WK[db * 128:(db + 1) * 128,
                                       fcol:fcol + 128].bitcast(F32R))
                                wkt.append(w_t)
                                w_t = wst.tile([128, 128], F32R,
                                               tag=f"wq{db}", name=f"wq{db}")
                                nc.sync.dma_start(
                                    w_t[:],
                                    WQ[db * 128:(db + 1) * 128,
                                       fcol:fcol + 128].bitcast(F32R))
                                wqt.append(w_t)
                            for tg in range(4):
                                ps = psKV.tile([128, 512], F32, tag="pk")
                                for db in range(8):
                                    nc.tensor.matmul(
                                        ps[:], wkt[db][:],
                                        hT[db][:, tg * 512:(tg + 1) * 512],
                                        start=(db == 0), stop=(db == 7))
                                nc.vector.tensor_scalar_add(
                                    out=kT[p][:, tg * 512:(tg + 1) * 512],
                                    in0=ps[:], scalar1=bk_c[:])
                            for tg in range(2):
                                ps = psKV.tile([128, 512], F32, tag="pk")
                                for db in range(8):
                                    nc.tensor.matmul(
                                        ps[:], wqt[db][:],
                                        hT[db][:, OWN + tg * 512:
                                               OWN + (tg + 1) * 512],
                                        start=(db == 0), stop=(db == 7))
                                nc.vector.tensor_scalar_add(
                                    out=qT[p][:, tg * 512:(tg + 1) * 512],
                                    in0=ps[:], scalar1=bq_c[:])

                        wvt = []
                        for db in range(8):
                            w_t = wst.tile([128, HPS * HD], F32R,
                                           tag=f"wv{db}", name=f"wv{db}")
                            nc.sync.dma_start(
                                w_t[:],
                                WV[db * 128:(db + 1) * 128,
                                   hs * 256:(hs + 1) * 256].bitcast(F32R))
                            wvt.append(w_t)
                        bv_t = wst.tile([1, HPS * HD], F32R, tag="bv")
                        nc.sync.dma_start(
                            bv_t[:],
                            BV[0:1, hs * 256:(hs + 1) * 256].bitcast(F32R))
                        for tb in range(SB):
                            ps = psKV.tile([128, HPS * HD], F32, tag="pv")
                            for db in range(8):
                                nc.tensor.matmul(
                                    ps[:], hT[db][:, tb * 128:(tb + 1) * 128],
                                    wvt[db][:], start=(db == 0), stop=False)
                            nc.tensor.matmul(ps[:], ones_r[:], bv_t[:],
                                             start=False, stop=True)
                            nc.vector.tensor_copy(
                                vS[tb][:, :, 0:HD],
                                ps[:].rearrange("p (h d) -> p h d", d=HD))
                            nc.scalar.copy(vS[tb][:, :, HD:HD + 1],
                                           ones_c3[:])

                        # ---- Phase 3: attention for this head set ----
                        for g in range(NQG):
                            kl = _klist(g)
                            # masked k-tiles for this q-group -> MSK index
                            mrel = {2 * g: 0, 2 * g + 1: 1,
                                    8 + 2 * g: 2, 9 + 2 * g: 3}
                            for h in range(HPS):
                                p, sub = h // 2, h % 2
                                pa = psA.tile([HD + 1, QG], F32, tag="pa")
                                nchunks = (len(kl) + CH - 1) // CH
                                for c in range(nchunks):
                                    chunk = kl[c * CH:(c + 1) * CH]
                                    wide = att.tile([128, CH * QG], F32R,
                                                    tag="wide", name="wide")
                                    if am_zero:
                                        for pi in range(len(chunk) // 2):
                                            kb0 = chunk[2 * pi]
                                            pss = psS.tile([128, 2 * QG], F32,
                                                           tag="ps")
                                            for u in range(2):
                                                kb = chunk[2 * pi + u]
                                                nc.tensor.matmul(
                                                    pss[:, u * QG:
                                                        (u + 1) * QG],
                                                    kT[p][sub * 64:
                                                          (sub + 1) * 64,
                                                          kb * 128:
                                                          (kb + 1) * 128],
                                                    qT[p][sub * 64:
                                                          (sub + 1) * 64,
                                                          g * QG:(g + 1) * QG],
                                                    start=True, stop=True)
                                            ws = wide[:, 2 * pi * QG:
                                                      (2 * pi + 2) * QG]
                                            if kb0 == 2 * g:
                                                nc.vector.scalar_tensor_tensor(
                                                    out=ws, in0=pss[:],
                                                    scalar=0.0, in1=mskB[:],
                                                    op0=ALU.add, op1=ALU.add)
                                                nc.scalar.activation(
                                                    ws, ws.bitcast(F32),
                                                    AF.Exp)
                                            elif kb0 == 8 + 2 * g:
                                                nc.vector.scalar_tensor_tensor(
                                                    out=ws, in0=pss[:],
                                                    scalar=0.0, in1=mskD[:],
                                                    op0=ALU.add, op1=ALU.add)
                                                nc.scalar.activation(
                                                    ws, ws.bitcast(F32),
                                                    AF.Exp)
                                            else:
                                                nc.scalar.activation(
                                                    ws, pss[:], AF.Exp)
                                    else:
                                        for i, kb in enumerate(chunk):
                                            pss = psS.tile([128, QG], F32,
                                                           tag="ps")
                                            nc.tensor.matmul(
                                                pss[:],
                                                kT[p][sub * 64:(sub + 1) * 64,
                                                      kb * 128:
                                                      (kb + 1) * 128],
                                                qT[p][sub * 64:(sub + 1) * 64,
                                                      g * QG:(g + 1) * QG],
                                                start=True, stop=True)
                                            wslice = wide[:, i * QG:
                                                          (i + 1) * QG]
                                            if kb in mrel:
                                                nc.vector.scalar_tensor_tensor(
                                                    out=wslice, in0=pss[:],
                                                    scalar=am_sb[:, kb:kb + 1],
                                                    in1=msk_t[g * 4
                                                              + mrel[kb]][:],
                                                    op0=ALU.add, op1=ALU.add)
                                            else:
                                                nc.vector.tensor_scalar_add(
                                                    out=wslice, in0=pss[:],
                                                    scalar1=am_sb[:,
                                                                  kb:kb + 1])
                                    if not am_zero:
                                        nw = len(chunk) * QG
                                        nc.scalar.activation(
                                            wide[:, 0:nw],
                                            wide[:, 0:nw].bitcast(F32),
                                            AF.Exp)
                                    for i, kb in enumerate(chunk):
                                        nc.tensor.matmul(
                                            pa[:], vS[kb][:, h, :],
                                            wide[:, i * QG:(i + 1) * QG],
                                            start=(c == 0 and i == 0),
                                            stop=(c == nchunks - 1
                                                  and i == len(chunk) - 1))
                                rec = att.tile([1, QG], F32R, tag="rec")
                                with nc.allow_low_precision(
                                        reason="softmax denom reciprocal, "
                                               "f32r is ~fp32"):
                                    nc.vector.reciprocal(rec[:],
                                                         pa[HD:HD + 1, :])
                                pb = psS.tile([64, QG], F32, tag="ps")
                                nc.tensor.matmul(pb[:], ones_b[:], rec[:],
                                                 start=True, stop=True)
                                bc = att.tile([64, QG], F32, tag="bc")
                                nc.scalar.copy(bc[:], pb[:])
                                ap_idx = 2 * hs + p
                                nc.vector.tensor_mul(
                                    aT[ap_idx][sub * 64:(sub + 1) * 64,
                                               g * QG:(g + 1) * QG],
                                    pa[0:HD, :], bc[:])

            if PL < 4:
                return
            # ---- Phase 4: proj + residual -> x2 ----
            x2p = top.enter_context(tc.tile_pool(name="x2p", bufs=1,
                                                 side="right"))
            x2 = [x2p.tile([128, D], F32, tag=f"x2{tb}", name=f"x2{tb}")
                  for tb in range(OB)]
            with tc.tile_pool(name="psP", bufs=2, space="PSUM") as psP, \
                 tc.tile_pool(name="wstP", bufs=1) as wst, \
                 tc.tile_pool(name="xqp", bufs=1) as xqp:
                xqs = []
                for tb in range(OB):
                    xq_t = xqp.tile([128, D], F32, tag=f"xq{tb}",
                                    name=f"xq{tb}")
                    nc.sync.dma_start(
                        xq_t[:], X[OWN + tb * 128:OWN + (tb + 1) * 128, :])
                    xqs.append(xq_t)
                for fg in range(2):
                    wpt = []
                    for ab in range(8):
                        w_t = wst.tile([128, 512], F32R, tag=f"wp{ab}",
                                       name=f"wp{ab}")
                        nc.sync.dma_start(
                            w_t[:],
                            WP[ab * 128:(ab + 1) * 128,
                               fg * 512:(fg + 1) * 512].bitcast(F32R))
                        wpt.append(w_t)
                    bp_t = wst.tile([1, 512], F32R, tag="bp")
                    nc.sync.dma_start(
                        bp_t[:],
                        BP[0:1, fg * 512:(fg + 1) * 512].bitcast(F32R))
                    for tb in range(OB):
                        ps = psP.tile([128, 512], F32, tag="pp")
                        for ab in range(8):
                            nc.tensor.matmul(
                                ps[:], aT[ab][:, tb * 128:(tb + 1) * 128],
                                wpt[ab][:], start=(ab == 0), stop=False)
                        nc.tensor.matmul(ps[:], ones_r[:], bp_t[:],
                                         start=False, stop=True)
                        nc.vector.tensor_add(
                            x2[tb][:, fg * 512:(fg + 1) * 512], ps[:],
                            xqs[tb][:, fg * 512:(fg + 1) * 512])

        if PL < 5:
            return
        # ---- Phase 5: LN2 + transpose -> h2T ----
        with contextlib.ExitStack() as mlp_stack:
            ht2 = mlp_stack.enter_context(tc.tile_pool(name="ht2", bufs=1))
            h2T = [ht2.tile([128, OWN], F32R, tag=f"h2T{db}", name=f"h2T{db}")
                   for db in range(8)]
            with tc.tile_pool(name="psT2", bufs=4, space="PSUM") as psT2, \
                 tc.tile_pool(name="xin5", bufs=3) as xin:
                for tb in range(OB):
                    x_t = xin.tile([128, D], F32, tag="x1")
                    nc.vector.tensor_copy(x_t[:], x2[tb][:])
                    _layernorm_tile(nc, stat, x_t, eps_t)
                    for db in range(8):
                        pt = psT2.tile([128, 128], F32, tag="tp")
                        nc.tensor.transpose(
                            pt[:], x_t[:, db * 128:(db + 1) * 128], ident[:])
                        nc.vector.tensor_copy(
                            h2T[db][:, tb * 128:(tb + 1) * 128], pt[:])

            if PL < 6:
                return
            # ---- Phase 6: MLP ----
            with contextlib.ExitStack() as mlp2:
                gtp = mlp2.enter_context(tc.tile_pool(name="gtp", bufs=1))
                wst = mlp2.enter_context(tc.tile_pool(name="wstF", bufs=3))
                wst6 = mlp2.enter_context(tc.tile_pool(name="wstF6", bufs=8))
                outp = mlp2.enter_context(tc.tile_pool(name="outp", bufs=3))
                psF = mlp2.enter_context(
                    tc.tile_pool(name="psF", bufs=3, space="PSUM"))
                psO = mlp2.enter_context(
                    tc.tile_pool(name="psO", bufs=1, space="PSUM"))
                for tg in range(2):
                    gt = [gtp.tile([128, 512], F32R, tag=f"gt{j}",
                                   name=f"gt{j}") for j in range(32)]
                    for jj in range(8):      # groups of 4 dff blocks
                        slabs = []
                        for db in range(8):
                            w_t = wst.tile([128, 512], F32R, tag=f"wf{db}",
                                           name=f"wf{db}")
                            nc.sync.dma_start(
                                w_t[:],
                                WF[db * 128:(db + 1) * 128,
                                   jj * 512:(jj + 1) * 512].bitcast(F32R))
                            slabs.append(w_t)
                        for sj in range(4):
                            j = jj * 4 + sj
                            bf_c = stat.tile([128, 1], F32, tag="bfc")
                            nc.sync.dma_start(bf_c[:],
                                              BF[j * 128:(j + 1) * 128, :])
                            ps = psF.tile([128, 512], F32, tag="pf")
                            for db in range(8):
                                nc.tensor.matmul(
                                    ps[:],
                                    slabs[db][:, sj * 128:(sj + 1) * 128],
                                    h2T[db][:, tg * 512:(tg + 1) * 512],
                                    start=(db == 0), stop=(db == 7))
                            nc.scalar.activation(gt[j][:], ps[:],
                                                 AF.Gelu_apprx_tanh,
                                                 bias=bf_c[:], scale=1.0)
                    for fg in range(2):
                        pso = [psO.tile([128, 512], F32, tag=f"po{tb}",
                                        name=f"po{tb}") for tb in range(4)]
                        for j in range(32):
                            w_t = wst6.tile([128, 512], F32R, tag="wf2",
                                            name="wf2")
                            nc.sync.dma_start(
                                w_t[:],
                                WF2[j * 128:(j + 1) * 128,
                                    fg * 512:(fg + 1) * 512].bitcast(F32R))
                            for tb in range(4):
                                nc.tensor.matmul(
                                    pso[tb][:],
                                    gt[j][:, tb * 128:(tb + 1) * 128],
                                    w_t[:], start=(j == 0), stop=False)
                        bf2_t = wst.tile([1, 512], F32R, tag="bf2")
                        nc.sync.dma_start(
                            bf2_t[:],
                            BF2[0:1, fg * 512:(fg + 1) * 512].bitcast(F32R))
                        for tb in range(4):
                            nc.tensor.matmul(pso[tb][:], ones_r[:], bf2_t[:],
                                             start=False, stop=True)
                            o_t = outp.tile([128, 512], F32, tag="ot")
                            gtb = tg * 4 + tb
                            nc.vector.tensor_add(
                                o_t[:], pso[tb][:],
                                x2[gtb][:, fg * 512:(fg + 1) * 512])
                            nc.sync.dma_start(
                                OUT[gtb * 128:(gtb + 1) * 128,
                                    fg * 512:(fg + 1) * 512], o_t[:])


_NC_CACHE = {}


def _get_nc(am_zero=True):
    key = f"nc{int(am_zero)}"
    if key not in _NC_CACHE:
        _NC_CACHE[key] = build_nc(am_zero)
    return _NC_CACHE[key]


def _perm_for(f):
    other = [2 * j + (1 - f) for j in range(8)]
    own = [2 * j + f for j in range(8)]
    blocks = other + own
    return np.concatenate([np.arange(b * 128, (b + 1) * 128) for b in blocks])


def make_in_maps(hidden_states, attention_mask, ln1_g, ln1_b, W_attn, b_attn,
                 W_proj, b_proj, ln2_g, ln2_b, W_fc, b_fc, W_fc2, b_fc2):
    f32 = lambda a: np.asarray(a, dtype=np.float32)
    hidden_states = f32(hidden_states)
    attention_mask = f32(attention_mask)
    ln1_g, ln1_b = f32(ln1_g), f32(ln1_b)
    ln2_g, ln2_b = f32(ln2_g), f32(ln2_b)
    W_attn, b_attn = f32(W_attn), f32(b_attn)
    W_proj, b_proj = f32(W_proj), f32(b_proj)
    W_fc, b_fc = f32(W_fc), f32(b_fc)
    W_fc2, b_fc2 = f32(W_fc2), f32(b_fc2)

    # Fold LN affines into the consuming matmuls (exact algebra, fp64 on host).
    Wa_eff = (ln1_g.astype(np.float64)[:, None] * W_attn).astype(np.float32)
    ba_eff = (b_attn.astype(np.float64)
              + ln1_b.astype(np.float64) @ W_attn).astype(np.float32)
    scale = 1.0 / np.sqrt(np.float32(HD))
    WQn = (Wa_eff[:, 0:D] * scale).astype(np.float32)
    BQn = (ba_eff[0:D] * scale).astype(np.float32)
    WKn, BKn = Wa_eff[:, D:2 * D].copy(), ba_eff[D:2 * D].copy()
    WVn, BVn = Wa_eff[:, 2 * D:3 * D].copy(), ba_eff[2 * D:3 * D].copy()
    Wf_eff = (ln2_g.astype(np.float64)[:, None] * W_fc).astype(np.float32)
    bf_eff = (b_fc.astype(np.float64)
              + ln2_b.astype(np.float64) @ W_fc).astype(np.float32)

    shared = {
        "WQ": np.ascontiguousarray(WQn),
        "WK": np.ascontiguousarray(WKn),
        "WV": np.ascontiguousarray(WVn),
        "BQ": np.ascontiguousarray(BQn[:, None]),
        "BK": np.ascontiguousarray(BKn[:, None]),
        "BV": np.ascontiguousarray(BVn[None, :]),
        "WP": np.ascontiguousarray(W_proj),
        "BP": np.ascontiguousarray(b_proj[None, :]),
        "WF": np.ascontiguousarray(Wf_eff),
        "BF": np.ascontiguousarray(bf_eff[:, None]),
        "WF2": np.ascontiguousarray(W_fc2),
        "BF2": np.ascontiguousarray(b_fc2[None, :]),
    }

    in_maps, perms = [], []
    for c in range(N_CORES):
        b, f = c >> 1, c & 1
        perm = _perm_for(f)
        perms.append(perm)
        x_ctx = np.ascontiguousarray(hidden_states[b][perm])
        gk = perm
        gq = perm[OWN:]
        causal = np.where(gk[:, None] <= gq[None, :], np.float32(0.0),
                          np.float32(MASKED_BIAS))
        am = attention_mask[b, 0, 0, :].astype(np.float32)
        am_zero = bool(np.all(attention_mask == 0))
        if am_zero:
            # pair tiles: [:, u*QG:(u+1)*QG] is k-block (base+u) vs q-group g
            # boundary pair (other-parity k blocks 2g, 2g+1) is g-independent
            msk = np.zeros((2, 128, 512), np.float32)
            g = 0
            for u, j in enumerate([2 * g, 2 * g + 1]):
                msk[0, :, u * QG:(u + 1) * QG] = causal[
                    j * 128:(j + 1) * 128, g * QG:(g + 1) * QG]
            for u, j in enumerate([8 + 2 * g, 9 + 2 * g]):
                msk[1, :, u * QG:(u + 1) * QG] = causal[
                    j * 128:(j + 1) * 128, g * QG:(g + 1) * QG]
        else:
            msk = np.empty((16, 128, QG), np.float32)
            for g in range(NQG):
                for rel, j in enumerate([2 * g, 2 * g + 1,
                                         8 + 2 * g, 9 + 2 * g]):
                    msk[g * 4 + rel] = causal[j * 128:(j + 1) * 128,
                                              g * QG:(g + 1) * QG]
        am_t = np.ascontiguousarray(am[perm].reshape(SB, 128).T)
        in_maps.append({"X": x_ctx, "MSK": np.ascontiguousarray(msk),
                        "AM": am_t, **shared})
    return in_maps, perms


def kernel(hidden_states, attention_mask, ln1_g, ln1_b, W_attn, b_attn,
           W_proj, b_proj, ln2_g, ln2_b, W_fc, b_fc, W_fc2, b_fc2):
    in_maps, perms = make_in_maps(
        hidden_states, attention_mask, ln1_g, ln1_b, W_attn, b_attn,
        W_proj, b_proj, ln2_g, ln2_b, W_fc, b_fc, W_fc2, b_fc2)
    am_zero = bool(np.all(np.asarray(attention_mask) == 0))
    nc = _get_nc(am_zero)
    res = run_bass_kernel_spmd(nc, in_maps, core_ids=list(range(N_CORES)))
    out = np.empty((B, S, D), dtype=np.float32)
    for c in range(N_CORES):
        b = c >> 1
        out[b][perms[c][OWN:]] = res.results[c]["OUT"]
    return out



# revision 33
# speedup vs baseline: 1.4395x; 1.4395x over previous
"""Fused GPT-2 transformer block on 8 Trainium2 NeuronCores.

Sharding: 8 cores = 4 batches x 2 causal-balanced folds. Core (b, f) owns the 8
interleaved 128-token blocks of parity f of batch b (queries), and receives all
2048 tokens of batch b as context, permuted [other-parity blocks | own blocks].
Causality is enforced by exact 0/1 mask multiplies after exp, so a single SPMD
program serves all cores. No collectives.

Layouts: LN1(x) is PE-transposed to hT [D, tok] (bf16); Q/K are produced in
head-major transposed layout (bf16), V in token-major layout with an appended
ones column (so the P@V matmul also accumulates softmax denominators).
Exp runs on the scalar engine in [128,1024] slabs straight from PSUM to bf16;
causal masking is a 0/1 elementwise multiply on the vector engine afterwards
(exp(s+m) == exp(s)*exp(m) with exp(m) in {0,1} exactly). proj/fc matmuls
contract against feature-major lhsT slices. All weights travel as bf16;
LN affines and the proj bias are folded on the host.
"""

import contextlib
import os

import numpy as np
import ml_dtypes

import concourse.bass as bass
import concourse.mybir as mybir
import concourse.tile as tile
from concourse import bacc
from concourse.bass_utils import run_bass_kernel_spmd
from concourse.masks import make_identity

F32 = mybir.dt.float32
F32R = mybir.dt.float32r
BF16 = mybir.dt.bfloat16
AF = mybir.ActivationFunctionType
ALU = mybir.AluOpType

B, S, D, H = 4, 2048, 1024, 16
HD = D // H          # 64
DFF = 4 * D          # 4096
EPS = 1e-5
MASKED_BIAS = -10000.0
N_CORES = 8

SB = S // 128        # 16 ctx blocks
OWN = S // 2         # 1024 own tokens
OB = OWN // 128      # 8 own blocks
NQG = 4              # q-groups of 256
QG = 256
HSETS = 4            # head sets
HPS = H // HSETS     # 4 heads per set


def _klist(g):
    """ctx k-block indices computed for q-group g (own blocks 2g, 2g+1)."""
    return list(range(0, 2 * g + 2)) + list(range(8, 8 + 2 * g + 2))


def build_nc(am_zero=True):
    nc = bacc.Bacc("TRN2", target_bir_lowering=False, debug=False,
                   num_devices=N_CORES)

    X = nc.dram_tensor("X", [S, D], BF16, kind="ExternalInput")
    XQ = nc.dram_tensor("XQ", [OWN, D], F32, kind="ExternalInput")
    MSKE = nc.dram_tensor("MSKE", [2, 128, 512], BF16, kind="ExternalInput")
    EAM = (None if am_zero else
           nc.dram_tensor("EAM", [128, SB], F32, kind="ExternalInput"))
    WQ = nc.dram_tensor("WQ", [D, D], BF16, kind="ExternalInput")
    WK = nc.dram_tensor("WK", [D, D], BF16, kind="ExternalInput")
    WV = nc.dram_tensor("WV", [D, D], BF16, kind="ExternalInput")
    BQ = nc.dram_tensor("BQ", [128, 8], F32, kind="ExternalInput")
    BK = nc.dram_tensor("BK", [128, 8], F32, kind="ExternalInput")
    BV = nc.dram_tensor("BV", [1, D], F32, kind="ExternalInput")
    WP = nc.dram_tensor("WP", [D, D], BF16, kind="ExternalInput")
    WF = nc.dram_tensor("WF", [D, DFF], BF16, kind="ExternalInput")
    BF = nc.dram_tensor("BF", [128, 32], F32, kind="ExternalInput")
    WF2 = nc.dram_tensor("WF2", [DFF, D], BF16, kind="ExternalInput")
    BF2 = nc.dram_tensor("BF2", [1, D], F32, kind="ExternalInput")
    OUT = nc.dram_tensor("OUT", [OWN, D], F32, kind="ExternalOutput")

    with tile.TileContext(nc) as tc:
        _body(nc, tc, X, XQ, MSKE, EAM, WQ, WK, WV, BQ, BK, BV, WP, WF, BF,
              WF2, BF2, OUT, am_zero)
    nc.compile()
    return nc


def _ln_stats(nc, stat, src, eps_t):
    """LN stats of src [128, D] -> (rinv [128,1], nb [128,1]) with
    nb = -mean * rinv."""
    sub = 512
    nsub = D // sub
    xs = src.rearrange("p (n s) -> p n s", s=sub)
    stats = stat.tile([128, nsub, nc.vector.BN_STATS_DIM], F32, tag="bnst")
    for j in range(nsub):
        nc.vector.bn_stats(out=stats[:, j, :], in_=xs[:, j, :])
    mv = stat.tile([128, nc.vector.BN_AGGR_DIM], F32, tag="bnag")
    nc.vector.bn_aggr(out=mv[:, :], in_=stats[:, :, :])
    nc.scalar.activation(out=mv[:, 1:2], in_=mv[:, 1:2], func=AF.Sqrt,
                         bias=eps_t[:], scale=1.0)
    rinv = stat.tile([128, 1], F32, tag="rinv")
    nc.vector.reciprocal(out=rinv[:], in_=mv[:, 1:2])
    nb = stat.tile([128, 1], F32, tag="nb")
    nc.vector.scalar_tensor_tensor(out=nb[:], in0=mv[:, 0:1], scalar=-1.0,
                                   in1=rinv[:], op0=ALU.mult, op1=ALU.mult)
    return rinv, nb


def _body(nc, tc, X, XQ, MSKE, EAM, WQ, WK, WV, BQ, BK, BV, WP, WF, BF,
          WF2, BF2, OUT, am_zero=True):
    PL = int(os.environ.get("KPHASES", "9"))
    with contextlib.ExitStack() as top:
        cst = top.enter_context(tc.tile_pool(name="cst", bufs=1))
        stat = top.enter_context(tc.tile_pool(name="stat", bufs=4))

        ident = cst.tile([128, 128], F32)
        make_identity(nc, ident[:])
        ones_f = cst.tile([1, 128], F32)
        nc.vector.memset(ones_f[:], 1.0)
        ones_c3 = cst.tile([128, HPS, 1], BF16)
        nc.vector.memset(ones_c3[:], 1.0)
        ones_r = cst.tile([1, 128], F32R)   # bias-row lhsT
        nc.scalar.copy(ones_r[:], ones_f[:])
        ones_b = cst.tile([1, 64], F32R)    # denominator-broadcast lhsT
        nc.scalar.copy(ones_b[:], ones_f[:, 0:64])
        eps_t = cst.tile([128, 1], F32)
        nc.vector.memset(eps_t[:], EPS)
        ident_b = cst.tile([128, 128], BF16)
        nc.scalar.copy(ident_b[:], ident[:])

        with contextlib.ExitStack() as attn_stack:
            atp = attn_stack.enter_context(tc.tile_pool(name="atp", bufs=1))
            aT = [atp.tile([128, OWN], BF16, tag=f"aT{p}", name=f"aT{p}")
                  for p in range(8)]

            with contextlib.ExitStack() as ht_stack:
                ht = ht_stack.enter_context(tc.tile_pool(name="ht", bufs=1))
                # hT[db][tg] : [128, 512] bf16, feature-major LN1(x)
                hT = [[ht.tile([128, 512], BF16, tag=f"hT{db}_{tg}",
                               name=f"hT{db}_{tg}") for tg in range(4)]
                      for db in range(8)]

                with contextlib.ExitStack() as hs_stack:
                    # attention-phase pools are created BEFORE the phase-1
                    # pools so phase-1 buffer teardown never aliases them
                    kvq = hs_stack.enter_context(
                        tc.tile_pool(name="kvq", bufs=2))
                    mskp = hs_stack.enter_context(
                        tc.tile_pool(name="mskp", bufs=1))
                    att = hs_stack.enter_context(
                        tc.tile_pool(name="att", bufs=3))
                    wst = hs_stack.enter_context(
                        tc.tile_pool(name="wstA", bufs=2))
                    psKV = hs_stack.enter_context(
                        tc.tile_pool(name="psKV", bufs=2, space="PSUM"))
                    psS = hs_stack.enter_context(
                        tc.tile_pool(name="psS", bufs=2, space="PSUM"))

                    # first X chunk goes to the head of the DMA queue so
                    # LN1 starts as early as possible
                    x_first = mskp.tile([128, 2, D], BF16, tag="xf",
                                        name="xf")
                    nc.sync.dma_start(
                        x_first[:],
                        X[0:256, :].rearrange("(i p) d -> p i d", p=128))

                    # 0/1 exp-masks (bf16) + per-token exp(attn-mask)
                    mskE = mskp.tile([128, 2, 512], BF16, tag="mskE",
                                     name="mskE")
                    nc.sync.dma_start(mskE[:],
                                      MSKE[:, :, :].rearrange("m p f -> p m f"))
                    eam = None
                    if not am_zero:
                        eam = mskp.tile([128, SB], F32, tag="eam", name="eam")
                        nc.sync.dma_start(eam[:], EAM[:, :])
                    bq_t = mskp.tile([128, 8], F32, tag="bq", name="bq")
                    nc.sync.dma_start(bq_t[:], BQ[:, :])
                    bk_t = mskp.tile([128, 8], F32, tag="bk", name="bk")
                    nc.sync.dma_start(bk_t[:], BK[:, :])
                    bv_t = mskp.tile([1, D], F32R, tag="bv", name="bv")
                    nc.sync.dma_start(bv_t[:], BV[:, :].bitcast(F32R))

                    # ---- Phase 1: LN1 over ctx + transpose -> hT ----
                    # 4 token-blocks transpose into one PSUM bank, so each
                    # hT[db][tg] tile is produced by a single wide copy.
                    with tc.tile_pool(name="psT", bufs=2, space="PSUM") \
                            as psT, \
                         tc.tile_pool(name="xin1", bufs=4) as xin, \
                         tc.tile_pool(name="xln", bufs=6) as xlnp:
                        for xg in range(4):
                            xts = []
                            for xh in range(2):
                                if xg == 0 and xh == 0:
                                    xts.append(x_first)
                                    continue
                                x_t = xin.tile([128, 2, D], BF16, tag="x1")
                                nc.sync.dma_start(
                                    x_t[:],
                                    X[xg * 512 + xh * 256:
                                      xg * 512 + (xh + 1) * 256,
                                      :].rearrange("(i p) d -> p i d",
                                                   p=128))
                                xts.append(x_t)
                            xls = []
                            for i in range(4):
                                xv = xts[i // 2][:, i % 2, :]
                                rinv, nb = _ln_stats(nc, stat, xv, eps_t)
                                x_ln = xlnp.tile([128, D], BF16, tag="xln")
                                nc.scalar.activation(out=x_ln[:], in_=xv,
                                                     func=AF.Identity,
                                                     bias=nb[:],
                                                     scale=rinv[:])
                                xls.append(x_ln)
                            for db in range(8):
                                pt = psT.tile([128, 512], BF16, tag="tp")
                                for i in range(4):
                                    nc.tensor.transpose(
                                        pt[:, i * 128:(i + 1) * 128],
                                        xls[i][:, db * 128:(db + 1) * 128],
                                        ident_b[:])
                                if db % 2 == 0:
                                    nc.vector.tensor_copy(hT[db][xg][:],
                                                          pt[:])
                                else:
                                    nc.scalar.copy(hT[db][xg][:], pt[:])

                    if PL < 2:
                        return
                    # psA reuses psT's freed banks; the region-reuse
                    # dependency (first pa write after last phase-1
                    # transpose copy) is subsumed by the data dependency
                    # attention -> K/V -> hT -> phase 1.
                    psA = hs_stack.enter_context(
                        tc.tile_pool(name="psA", bufs=2, space="PSUM"))
                    for hs in range(HSETS):
                        # ---- K/V/Q projections for this head set ----
                        kT = [kvq.tile([128, S], BF16, tag=f"kT{p}",
                                       name=f"kT{p}") for p in range(2)]
                        qT = [kvq.tile([128, OWN], BF16, tag=f"qT{p}",
                                       name=f"qT{p}") for p in range(2)]
                        vS = [kvq.tile([128, HPS, HD + 1], BF16,
                                       tag=f"vS{tb}", name=f"vS{tb}")
                              for tb in range(SB)]

                        wkq = []
                        for p in range(2):
                            fcol = hs * 256 + p * 128
                            wk_t = wst.tile([128, 8, 128], BF16,
                                            tag=f"wk{p}", name=f"wk{p}")
                            nc.sync.dma_start(
                                wk_t[:],
                                WK[:, fcol:fcol + 128].rearrange(
                                    "(i p2) f -> p2 i f", p2=128))
                            wq_t = wst.tile([128, 8, 128], BF16,
                                            tag=f"wq{p}", name=f"wq{p}")
                            nc.sync.dma_start(
                                wq_t[:],
                                WQ[:, fcol:fcol + 128].rearrange(
                                    "(i p2) f -> p2 i f", p2=128))
                            wkq.append((wk_t, wq_t))
                        wv_t = wst.tile([128, 8, 256], BF16, tag="wv",
                                        name="wv")
                        nc.sync.dma_start(
                            wv_t[:],
                            WV[:, hs * 256:(hs + 1) * 256].rearrange(
                                "(i p2) f -> p2 i f", p2=128))

                        for p in range(2):
                            wk_t, wq_t = wkq[p]
                            bcol = hs * 2 + p
                            for tg in range(4):
                                ps = psKV.tile([128, 512], F32, tag="pk")
                                for db in range(8):
                                    nc.tensor.matmul(
                                        ps[:], wk_t[:, db, :], hT[db][tg][:],
                                        start=(db == 0), stop=(db == 7))
                                nc.vector.tensor_scalar_add(
                                    out=kT[p][:, tg * 512:(tg + 1) * 512],
                                    in0=ps[:],
                                    scalar1=bk_t[:, bcol:bcol + 1])
                            for tg in range(2):
                                ps = psKV.tile([128, 512], F32, tag="pk")
                                for db in range(8):
                                    nc.tensor.matmul(
                                        ps[:], wq_t[:, db, :],
                                        hT[db][2 + tg][:],
                                        start=(db == 0), stop=(db == 7))
                                nc.vector.tensor_scalar_add(
                                    out=qT[p][:, tg * 512:(tg + 1) * 512],
                                    in0=ps[:],
                                    scalar1=bq_t[:, bcol:bcol + 1])

                        # V bias broadcast tile [128, 256] for this head set
                        psb = psKV.tile([128, 512], F32, tag="pk")
                        nc.tensor.matmul(
                            psb[:, 0:256], ones_r[:],
                            bv_t[0:1, hs * 256:(hs + 1) * 256],
                            start=True, stop=True)
                        bvb = att.tile([128, HPS, HD], F32, tag="bvb")
                        nc.scalar.copy(
                            bvb[:],
                            psb[:, 0:256].rearrange("p (h d) -> p h d", d=HD))

                        for tb in range(SB):
                            ps = psKV.tile([128, 512], F32, tag="pk")
                            for db in range(8):
                                nc.tensor.matmul(
                                    ps[:, 0:256],
                                    hT[db][tb // 4][:, (tb % 4) * 128:
                                                    (tb % 4 + 1) * 128],
                                    wv_t[:, db, :],
                                    start=(db == 0), stop=(db == 7))
                            nc.vector.tensor_tensor(
                                out=vS[tb][:, :, 0:HD],
                                in0=ps[:, 0:256].rearrange(
                                    "p (h d) -> p h d", d=HD),
                                in1=bvb[:], op=ALU.add)
                            nc.gpsimd.tensor_copy(vS[tb][:, :, HD:HD + 1],
                                                  ones_c3[:])

                        # ---- attention for this head set ----
                        # PV runs as P.T @ V: the exp block [128k, 128q] is
                        # the stationary operand and V [128k, 65] the bf16
                        # moving operand (65 rows/matmul). The ones column
                        # of V accumulates softmax denominators into the
                        # output's col 64, so normalization is a cheap
                        # per-partition scalar multiply; a PE transpose
                        # brings the normalized [q, feat] block back to
                        # feature-major aT for proj.
                        for g in range(NQG):
                            kl = _klist(g)
                            nquads = g + 1
                            for h in range(HPS):
                                p, sub = h // 2, h % 2
                                # one bank: q-sub accumulators at cols
                                # 0:65 / 128:193, transposed-normalized
                                # output at cols 256:512
                                pab = psA.tile([128, 2 * QG], F32, tag="pab")
                                pq = [pab[:, 0:HD + 1],
                                      pab[:, 128:128 + HD + 1]]
                                for qd in range(nquads):
                                    blocks = kl[4 * qd:4 * qd + 4]
                                    pss = psS.tile([128, 1024], F32,
                                                   tag="ps")
                                    for u in range(4):
                                        kb = blocks[u]
                                        nc.tensor.matmul(
                                            pss[:, u * QG:(u + 1) * QG],
                                            kT[p][sub * 64:(sub + 1) * 64,
                                                  kb * 128:(kb + 1) * 128],
                                            qT[p][sub * 64:(sub + 1) * 64,
                                                  g * QG:(g + 1) * QG],
                                            start=True, stop=True)
                                    wide = att.tile([128, 1024], BF16,
                                                    tag="wide", name="wide")
                                    nc.scalar.activation(wide[:], pss[:],
                                                         AF.Exp)
                                    if qd == g // 2:
                                        sl = wide[:, (g % 2) * 512:
                                                  (g % 2) * 512 + 512]
                                        nc.vector.tensor_mul(
                                            sl, sl, mskE[:, 0, :])
                                    if qd == g:
                                        sl = wide[:, 512:1024]
                                        nc.vector.tensor_mul(
                                            sl, sl, mskE[:, 1, :])
                                    if not am_zero:
                                        for u in range(4):
                                            kb = blocks[u]
                                            sl = wide[:, u * QG:(u + 1) * QG]
                                            nc.vector.tensor_scalar_mul(
                                                out=sl, in0=sl,
                                                scalar1=eam[:, kb:kb + 1])
                                    # one accumulation group for the whole
                                    # bank: start clears the bank-wide
                                    # has_written bits, so only the very
                                    # first matmul may carry it
                                    for u in range(4):
                                        kb = blocks[u]
                                        for qs in range(2):
                                            nc.tensor.matmul(
                                                pq[qs],
                                                wide[:, u * QG + qs * 128:
                                                     u * QG + qs * 128
                                                     + 128],
                                                vS[kb][:, h, :],
                                                start=(qd == 0 and u == 0
                                                       and qs == 0),
                                                stop=(qd == nquads - 1
                                                      and u == 3
                                                      and qs == 1),
                                                skip_group_check=True)
                                ap_idx = 2 * hs + p
                                for qs in range(2):
                                    rec = att.tile([128, 1], F32,
                                                   tag="rec")
                                    nc.vector.reciprocal(
                                        rec[:], pq[qs][:, HD:HD + 1])
                                    anrm = att.tile([128, HD], BF16,
                                                    tag="anrm")
                                    nc.vector.tensor_scalar_mul(
                                        out=anrm[:], in0=pq[qs][:, 0:HD],
                                        scalar1=rec[:])
                                    nc.tensor.transpose(
                                        pab[0:HD, QG + qs * 64:
                                            QG + (qs + 1) * 64].bitcast(
                                                BF16),
                                        anrm[:], ident_b[:])
                                dst = aT[ap_idx][sub * 64:(sub + 1) * 64,
                                                 g * QG:(g + 1) * QG]
                                src = pab[0:HD, QG:QG + 128].bitcast(BF16)
                                if h % 2 == 0:
                                    nc.vector.tensor_copy(dst, src)
                                else:
                                    nc.scalar.copy(dst, src)

            if PL < 4:
                return
            # ---- proj + residual -> x2 ; prefetch WF/BF/BF2 ----
            psT2 = top.enter_context(
                tc.tile_pool(name="psT2", bufs=2, space="PSUM"))
            psF = top.enter_context(
                tc.tile_pool(name="psF", bufs=2, space="PSUM"))
            x2p = top.enter_context(tc.tile_pool(name="x2p", bufs=1,
                                                 side="right"))
            wfp = top.enter_context(tc.tile_pool(name="wfp", bufs=1,
                                                 side="right"))
            wf_t = [wfp.tile([128, DFF], BF16, tag=f"wf{db}",
                             name=f"wf{db}") for db in range(8)]
            for db in range(8):
                nc.sync.dma_start(wf_t[db][:],
                                  WF[db * 128:(db + 1) * 128, :])
            bf_t = wfp.tile([128, 32], F32, tag="bf", name="bf")
            nc.sync.dma_start(bf_t[:], BF[:, :])
            bf2_t = wfp.tile([1, D], F32R, tag="bf2", name="bf2")
            nc.sync.dma_start(bf2_t[:], BF2[:, :].bitcast(F32R))

            x2 = [x2p.tile([128, 4, D], F32, tag=f"x2{i}", name=f"x2{i}")
                  for i in range(2)]
            for i in range(2):
                nc.sync.dma_start(
                    x2[i][:],
                    XQ[i * 512:(i + 1) * 512, :].rearrange(
                        "(i2 p) d -> p i2 d", p=128))

            def x2v(tb):
                return x2[tb // 4][:, tb % 4, :]

            with tc.tile_pool(name="psP", bufs=2, space="PSUM") as psP, \
                 tc.tile_pool(name="wstP", bufs=1) as wstp:
                wpt = []
                for fg in range(2):
                    w_t = wstp.tile([128, 8, 512], BF16, tag=f"wp{fg}",
                                    name=f"wp{fg}")
                    nc.sync.dma_start(
                        w_t[:],
                        WP[:, fg * 512:(fg + 1) * 512].rearrange(
                            "(i p2) f -> p2 i f", p2=128))
                    wpt.append(w_t)
                for tb in range(OB):
                    for fg in range(2):
                        ps = psP.tile([128, 512], F32, tag="pp")
                        for ab in range(8):
                            nc.tensor.matmul(
                                ps[:], aT[ab][:, tb * 128:(tb + 1) * 128],
                                wpt[fg][:, ab, :], start=(ab == 0),
                                stop=(ab == 7))
                        dst = x2v(tb)[:, fg * 512:(fg + 1) * 512]
                        nc.vector.tensor_tensor(out=dst, in0=dst, in1=ps[:],
                                                op=ALU.add)

        if PL < 5:
            return
        # ---- LN2 + transpose -> h2T ; then MLP ----
        with contextlib.ExitStack() as mlp_stack:
            ht2 = mlp_stack.enter_context(tc.tile_pool(name="ht2", bufs=1))
            h2T = [[ht2.tile([128, 512], BF16, tag=f"h2T{db}_{tg}",
                             name=f"h2T{db}_{tg}") for tg in range(2)]
                   for db in range(8)]
            with tc.tile_pool(name="xln2", bufs=6) as xlnp:
                for tg in range(2):
                    xls = []
                    for i in range(4):
                        tb = tg * 4 + i
                        rinv, nb = _ln_stats(nc, stat, x2v(tb), eps_t)
                        x_ln = xlnp.tile([128, D], BF16, tag="xln")
                        nc.scalar.activation(out=x_ln[:], in_=x2v(tb),
                                             func=AF.Identity,
                                             bias=nb[:], scale=rinv[:])
                        xls.append(x_ln)
                    for db in range(8):
                        pt = psT2.tile([128, 512], BF16, tag="tp")
                        for i in range(4):
                            nc.tensor.transpose(
                                pt[:, i * 128:(i + 1) * 128],
                                xls[i][:, db * 128:(db + 1) * 128],
                                ident_b[:])
                        if db % 2 == 0:
                            nc.vector.tensor_copy(h2T[db][tg][:], pt[:])
                        else:
                            nc.scalar.copy(h2T[db][tg][:], pt[:])

            if PL < 6:
                return
            with contextlib.ExitStack() as mlp2:
                gtp = mlp2.enter_context(tc.tile_pool(name="gtp", bufs=1))
                wst6 = mlp2.enter_context(tc.tile_pool(name="wstF6", bufs=3))
                outp = mlp2.enter_context(tc.tile_pool(name="outp", bufs=3))
                psO = None
                for tg in range(2):
                    gt = [gtp.tile([128, 512], BF16, tag=f"gt{j}",
                                   name=f"gt{j}") for j in range(32)]
                    for j in range(32):
                        ps = psF.tile([128, 512], F32, tag="pf")
                        for db in range(8):
                            nc.tensor.matmul(
                                ps[:], wf_t[db][:, j * 128:(j + 1) * 128],
                                h2T[db][tg][:],
                                start=(db == 0), stop=(db == 7))
                        nc.scalar.activation(gt[j][:], ps[:],
                                             AF.Gelu_apprx_tanh,
                                             bias=bf_t[:, j:j + 1], scale=1.0)
                    if psO is None:
                        psO = mlp2.enter_context(
                            tc.tile_pool(name="psO", bufs=1, space="PSUM"))
                    for fg in range(2):
                        pso = [psO.tile([128, 512], F32, tag=f"po{tb}",
                                        name=f"po{tb}") for tb in range(4)]
                        for jj in range(4):
                            w8 = wst6.tile([128, 8, 512], BF16, tag="wf2",
                                           name="wf2")
                            nc.sync.dma_start(
                                w8[:],
                                WF2[jj * 1024:(jj + 1) * 1024,
                                    fg * 512:(fg + 1) * 512].rearrange(
                                        "(i p2) f -> p2 i f", p2=128))
                            for jr in range(8):
                                j = jj * 8 + jr
                                for tb in range(4):
                                    nc.tensor.matmul(
                                        pso[tb][:],
                                        gt[j][:, tb * 128:(tb + 1) * 128],
                                        w8[:, jr, :], start=(j == 0),
                                        stop=False)
                        for tb in range(4):
                            nc.tensor.matmul(
                                pso[tb][:], ones_r[:],
                                bf2_t[0:1, fg * 512:(fg + 1) * 512],
                                start=False, stop=True)
                            gtb = tg * 4 + tb
                            o_t = outp.tile([128, 512], F32, tag="ot")
                            nc.vector.tensor_add(
                                o_t[:], pso[tb][:],
                                x2v(gtb)[:, fg * 512:(fg + 1) * 512])
                            nc.scalar.dma_start(
                                OUT[gtb * 128:(gtb + 1) * 128,
                                    fg * 512:(fg + 1) * 512], o_t[:])


_NC_CACHE = {}


def _get_nc(am_zero=True):
    key = f"nc{int(am_zero)}"
    if key not in _NC_CACHE:
        _NC_CACHE[key] = build_nc(am_zero)
    return _NC_CACHE[key]


def _perm_for(f):
    other = [2 * j + (1 - f) for j in range(8)]
    own = [2 * j + f for j in range(8)]
    blocks = other + own
    return np.concatenate([np.arange(b * 128, (b + 1) * 128) for b in blocks])


def make_in_maps(hidden_states, attention_mask, ln1_g, ln1_b, W_attn, b_attn,
                 W_proj, b_proj, ln2_g, ln2_b, W_fc, b_fc, W_fc2, b_fc2):
    f32 = lambda a: np.asarray(a, dtype=np.float32)
    bf16 = lambda a: np.ascontiguousarray(a.astype(ml_dtypes.bfloat16))
    hidden_states = f32(hidden_states)
    attention_mask = f32(attention_mask)
    ln1_g, ln1_b = f32(ln1_g), f32(ln1_b)
    ln2_g, ln2_b = f32(ln2_g), f32(ln2_b)
    W_attn, b_attn = f32(W_attn), f32(b_attn)
    W_proj, b_proj = f32(W_proj), f32(b_proj)
    W_fc, b_fc = f32(W_fc), f32(b_fc)
    W_fc2, b_fc2 = f32(W_fc2), f32(b_fc2)

    # Fold LN affines into the consuming matmuls (exact algebra, fp64 on host).
    Wa_eff = (ln1_g.astype(np.float64)[:, None] * W_attn).astype(np.float32)
    ba_eff = (b_attn.astype(np.float64)
              + ln1_b.astype(np.float64) @ W_attn).astype(np.float32)
    scale = 1.0 / np.sqrt(np.float32(HD))
    WQn = (Wa_eff[:, 0:D] * scale).astype(np.float32)
    BQn = (ba_eff[0:D] * scale).astype(np.float32)
    WKn, BKn = Wa_eff[:, D:2 * D].copy(), ba_eff[D:2 * D].copy()
    WVn, BVn = Wa_eff[:, 2 * D:3 * D].copy(), ba_eff[2 * D:3 * D].copy()
    Wf_eff = (ln2_g.astype(np.float64)[:, None] * W_fc).astype(np.float32)
    bf_eff = (b_fc.astype(np.float64)
              + ln2_b.astype(np.float64) @ W_fc).astype(np.float32)

    shared = {
        "WQ": bf16(WQn),
        "WK": bf16(WKn),
        "WV": bf16(WVn),
        "BQ": np.ascontiguousarray(BQn.reshape(8, 128).T),
        "BK": np.ascontiguousarray(BKn.reshape(8, 128).T),
        "BV": np.ascontiguousarray(BVn[None, :]),
        "WP": bf16(W_proj),
        "WF": bf16(Wf_eff),
        "BF": np.ascontiguousarray(bf_eff.reshape(32, 128).T),
        "WF2": bf16(W_fc2),
        "BF2": np.ascontiguousarray(b_fc2[None, :]),
    }

    am_zero = bool(np.all(attention_mask == 0))
    in_maps, perms = [], []
    for c in range(N_CORES):
        b, f = c >> 1, c & 1
        perm = _perm_for(f)
        perms.append(perm)
        x_ctx = np.ascontiguousarray(hidden_states[b][perm])
        xq = np.ascontiguousarray(hidden_states[b][perm[OWN:]]
                                  + b_proj[None, :])
        gk = perm
        gq = perm[OWN:]
        live = (gk[:, None] <= gq[None, :]).astype(np.float32)
        # 0/1 exp-masks: [:, u*QG:(u+1)*QG] is k-block (base+u) vs q-group 0
        # pair 0: other-parity blocks (0, 1); pair 1: own blocks (8, 9).
        # The relative pattern is g-independent.
        msk = np.zeros((2, 128, 512), np.float32)
        for u, j in enumerate([0, 1]):
            msk[0, :, u * QG:(u + 1) * QG] = live[
                j * 128:(j + 1) * 128, 0:QG]
        for u, j in enumerate([8, 9]):
            msk[1, :, u * QG:(u + 1) * QG] = live[
                j * 128:(j + 1) * 128, 0:QG]
        im = {"X": bf16(x_ctx), "XQ": xq, "MSKE": bf16(msk), **shared}
        if not am_zero:
            am = attention_mask[b, 0, 0, :].astype(np.float64)
            eam = np.exp(am[perm]).astype(np.float32)
            im["EAM"] = np.ascontiguousarray(eam.reshape(SB, 128).T)
        in_maps.append(im)
    return in_maps, perms, am_zero


def kernel(hidden_states, attention_mask, ln1_g, ln1_b, W_attn, b_attn,
           W_proj, b_proj, ln2_g, ln2_b, W_fc, b_fc, W_fc2, b_fc2):
    in_maps, perms, am_zero = make_in_maps(
        hidden_states, attention_mask, ln1_g, ln1_b, W_attn, b_attn,
        W_proj, b_proj, ln2_g, ln2_b, W_fc, b_fc, W_fc2, b_fc2)
    nc = _get_nc(am_zero)
    res = run_bass_kernel_spmd(nc, in_maps, core_ids=list(range(N_CORES)))
    out = np.empty((B, S, D), dtype=np.float32)
    for c in range(N_CORES):
        b = c >> 1
        out[b][perms[c][OWN:]] = res.results[c]["OUT"]
    return out


# revision 36
# speedup vs baseline: 1.4534x; 1.0097x over previous
"""Fused GPT-2 transformer block on 8 Trainium2 NeuronCores.

Sharding: 8 cores = 4 batches x 2 causal-balanced folds. Core (b, f) owns the 8
interleaved 128-token blocks of parity f of batch b (queries), and receives all
2048 tokens of batch b as context, permuted [other-parity blocks | own blocks].
Causality is enforced by exact 0/1 mask multiplies after exp, so a single SPMD
program serves all cores. No collectives.

Layouts: LN1(x) is PE-transposed to hT [D, tok] (bf16); Q/K are produced in
head-major transposed layout (bf16), V in token-major layout with an appended
ones column (so the P@V matmul also accumulates softmax denominators).
Exp runs on the scalar engine in [128,1024] slabs straight from PSUM to bf16;
causal masking is a 0/1 elementwise multiply on the vector engine afterwards
(exp(s+m) == exp(s)*exp(m) with exp(m) in {0,1} exactly). proj/fc matmuls
contract against feature-major lhsT slices. All weights travel as bf16;
LN affines and the proj bias are folded on the host.
"""

import contextlib
import os

import numpy as np
import ml_dtypes

import concourse.bass as bass
import concourse.mybir as mybir
import concourse.tile as tile
from concourse import bacc
from concourse.bass_utils import run_bass_kernel_spmd
from concourse.masks import make_identity

F32 = mybir.dt.float32
F32R = mybir.dt.float32r
BF16 = mybir.dt.bfloat16
AF = mybir.ActivationFunctionType
ALU = mybir.AluOpType

B, S, D, H = 4, 2048, 1024, 16
HD = D // H          # 64
DFF = 4 * D          # 4096
EPS = 1e-5
MASKED_BIAS = -10000.0
N_CORES = 8

SB = S // 128        # 16 ctx blocks
OWN = S // 2         # 1024 own tokens
OB = OWN // 128      # 8 own blocks
NQG = 4              # q-groups of 256
QG = 256
HSETS = 4            # head sets
HPS = H // HSETS     # 4 heads per set


def _klist(g):
    """ctx k-block indices computed for q-group g (own blocks 2g, 2g+1)."""
    return list(range(0, 2 * g + 2)) + list(range(8, 8 + 2 * g + 2))


def build_nc(am_zero=True):
    nc = bacc.Bacc("TRN2", target_bir_lowering=False, debug=False,
                   num_devices=N_CORES)

    X = nc.dram_tensor("X", [S, D], BF16, kind="ExternalInput")
    XQ = nc.dram_tensor("XQ", [OWN, D], F32, kind="ExternalInput")
    MSKE = nc.dram_tensor("MSKE", [2, 128, 512], BF16, kind="ExternalInput")
    EAM = (None if am_zero else
           nc.dram_tensor("EAM", [128, SB], F32, kind="ExternalInput"))
    WQ = nc.dram_tensor("WQ", [D, D], BF16, kind="ExternalInput")
    WK = nc.dram_tensor("WK", [D, D], BF16, kind="ExternalInput")
    WV = nc.dram_tensor("WV", [D, D], BF16, kind="ExternalInput")
    BQ = nc.dram_tensor("BQ", [128, 8], F32, kind="ExternalInput")
    BK = nc.dram_tensor("BK", [128, 8], F32, kind="ExternalInput")
    BV = nc.dram_tensor("BV", [1, D], F32, kind="ExternalInput")
    WP = nc.dram_tensor("WP", [D, D], BF16, kind="ExternalInput")
    WF = nc.dram_tensor("WF", [D, DFF], BF16, kind="ExternalInput")
    BF = nc.dram_tensor("BF", [128, 32], F32, kind="ExternalInput")
    WF2 = nc.dram_tensor("WF2", [DFF, D], BF16, kind="ExternalInput")
    BF2 = nc.dram_tensor("BF2", [1, D], F32, kind="ExternalInput")
    OUT = nc.dram_tensor("OUT", [OWN, D], F32, kind="ExternalOutput")

    with tile.TileContext(nc) as tc:
        _body(nc, tc, X, XQ, MSKE, EAM, WQ, WK, WV, BQ, BK, BV, WP, WF, BF,
              WF2, BF2, OUT, am_zero)
    nc.compile()
    return nc


def _ln_stats(nc, stat, src, eps_t):
    """LN stats of src [128, D] -> (rinv [128,1], nb [128,1]) with
    nb = -mean * rinv."""
    sub = 512
    nsub = D // sub
    xs = src.rearrange("p (n s) -> p n s", s=sub)
    stats = stat.tile([128, nsub, nc.vector.BN_STATS_DIM], F32, tag="bnst")
    for j in range(nsub):
        nc.vector.bn_stats(out=stats[:, j, :], in_=xs[:, j, :])
    mv = stat.tile([128, nc.vector.BN_AGGR_DIM], F32, tag="bnag")
    nc.vector.bn_aggr(out=mv[:, :], in_=stats[:, :, :])
    nc.scalar.activation(out=mv[:, 1:2], in_=mv[:, 1:2], func=AF.Sqrt,
                         bias=eps_t[:], scale=1.0)
    rinv = stat.tile([128, 1], F32, tag="rinv")
    nc.vector.reciprocal(out=rinv[:], in_=mv[:, 1:2])
    nb = stat.tile([128, 1], F32, tag="nb")
    nc.vector.scalar_tensor_tensor(out=nb[:], in0=mv[:, 0:1], scalar=-1.0,
                                   in1=rinv[:], op0=ALU.mult, op1=ALU.mult)
    return rinv, nb


def _body(nc, tc, X, XQ, MSKE, EAM, WQ, WK, WV, BQ, BK, BV, WP, WF, BF,
          WF2, BF2, OUT, am_zero=True):
    PL = int(os.environ.get("KPHASES", "9"))
    with contextlib.ExitStack() as top:
        cst = top.enter_context(tc.tile_pool(name="cst", bufs=1))
        stat = top.enter_context(tc.tile_pool(name="stat", bufs=4))

        ident = cst.tile([128, 128], F32)
        make_identity(nc, ident[:])
        ones_f = cst.tile([1, 128], F32)
        nc.vector.memset(ones_f[:], 1.0)
        ones_c3 = cst.tile([128, HPS, 1], BF16)
        nc.vector.memset(ones_c3[:], 1.0)
        ones_r = cst.tile([1, 128], F32R)   # bias-row lhsT
        nc.scalar.copy(ones_r[:], ones_f[:])
        ones_b = cst.tile([1, 64], F32R)    # denominator-broadcast lhsT
        nc.scalar.copy(ones_b[:], ones_f[:, 0:64])
        eps_t = cst.tile([128, 1], F32)
        nc.vector.memset(eps_t[:], EPS)
        ident_b = cst.tile([128, 128], BF16)
        nc.scalar.copy(ident_b[:], ident[:])

        with contextlib.ExitStack() as attn_stack:
            atp = attn_stack.enter_context(tc.tile_pool(name="atp", bufs=1))
            aT = [atp.tile([128, OWN], BF16, tag=f"aT{p}", name=f"aT{p}")
                  for p in range(8)]

            with contextlib.ExitStack() as ht_stack:
                ht = ht_stack.enter_context(tc.tile_pool(name="ht", bufs=1))
                # hT[db][tg] : [128, 512] bf16, feature-major LN1(x)
                hT = [[ht.tile([128, 512], BF16, tag=f"hT{db}_{tg}",
                               name=f"hT{db}_{tg}") for tg in range(4)]
                      for db in range(8)]

                with contextlib.ExitStack() as hs_stack:
                    # attention-phase pools are created BEFORE the phase-1
                    # pools so phase-1 buffer teardown never aliases them
                    kvq = hs_stack.enter_context(
                        tc.tile_pool(name="kvq", bufs=2))
                    mskp = hs_stack.enter_context(
                        tc.tile_pool(name="mskp", bufs=1))
                    att = hs_stack.enter_context(
                        tc.tile_pool(name="att", bufs=3))
                    wst = hs_stack.enter_context(
                        tc.tile_pool(name="wstA", bufs=2))
                    psKV = hs_stack.enter_context(
                        tc.tile_pool(name="psKV", bufs=2, space="PSUM"))
                    psS = hs_stack.enter_context(
                        tc.tile_pool(name="psS", bufs=2, space="PSUM"))

                    # first X chunk goes to the head of the DMA queue so
                    # LN1 starts as early as possible
                    x_first = mskp.tile([128, 2, D], BF16, tag="xf",
                                        name="xf")
                    nc.sync.dma_start(
                        x_first[:],
                        X[0:256, :].rearrange("(i p) d -> p i d", p=128))

                    # 0/1 exp-masks (bf16) + per-token exp(attn-mask)
                    mskE = mskp.tile([128, 2, 512], BF16, tag="mskE",
                                     name="mskE")
                    nc.sync.dma_start(mskE[:],
                                      MSKE[:, :, :].rearrange("m p f -> p m f"))
                    eam = None
                    if not am_zero:
                        eam = mskp.tile([128, SB], F32, tag="eam", name="eam")
                        nc.sync.dma_start(eam[:], EAM[:, :])
                    bq_t = mskp.tile([128, 8], F32, tag="bq", name="bq")
                    nc.sync.dma_start(bq_t[:], BQ[:, :])
                    bk_t = mskp.tile([128, 8], F32, tag="bk", name="bk")
                    nc.sync.dma_start(bk_t[:], BK[:, :])
                    bv_t = mskp.tile([1, D], F32R, tag="bv", name="bv")
                    nc.sync.dma_start(bv_t[:], BV[:, :].bitcast(F32R))

                    # ---- Phase 1: LN1 over ctx + transpose -> hT ----
                    # 4 token-blocks transpose into one PSUM bank, so each
                    # hT[db][tg] tile is produced by a single wide copy.
                    with tc.tile_pool(name="psT", bufs=2, space="PSUM") \
                            as psT, \
                         tc.tile_pool(name="xin1", bufs=4) as xin, \
                         tc.tile_pool(name="xln", bufs=6) as xlnp:
                        for xg in range(4):
                            xts = []
                            for xh in range(2):
                                if xg == 0 and xh == 0:
                                    xts.append(x_first)
                                    continue
                                x_t = xin.tile([128, 2, D], BF16, tag="x1")
                                nc.sync.dma_start(
                                    x_t[:],
                                    X[xg * 512 + xh * 256:
                                      xg * 512 + (xh + 1) * 256,
                                      :].rearrange("(i p) d -> p i d",
                                                   p=128))
                                xts.append(x_t)
                            xls = []
                            for i in range(4):
                                xv = xts[i // 2][:, i % 2, :]
                                rinv, nb = _ln_stats(nc, stat, xv, eps_t)
                                x_ln = xlnp.tile([128, D], BF16, tag="xln")
                                nc.scalar.activation(out=x_ln[:], in_=xv,
                                                     func=AF.Identity,
                                                     bias=nb[:],
                                                     scale=rinv[:])
                                xls.append(x_ln)
                            for db in range(8):
                                pt = psT.tile([128, 512], BF16, tag="tp")
                                for i in range(4):
                                    nc.tensor.transpose(
                                        pt[:, i * 128:(i + 1) * 128],
                                        xls[i][:, db * 128:(db + 1) * 128],
                                        ident_b[:])
                                if db % 2 == 0:
                                    nc.vector.tensor_copy(hT[db][xg][:],
                                                          pt[:])
                                else:
                                    nc.scalar.copy(hT[db][xg][:], pt[:])

                    if PL < 2:
                        return
                    # psA reuses psT's freed banks; the region-reuse
                    # dependency (first pa write after last phase-1
                    # transpose copy) is subsumed by the data dependency
                    # attention -> K/V -> hT -> phase 1.
                    psA = hs_stack.enter_context(
                        tc.tile_pool(name="psA", bufs=2, space="PSUM"))
                    for hs in range(HSETS):
                        # ---- K/V/Q projections for this head set ----
                        kT = [kvq.tile([128, S], BF16, tag=f"kT{p}",
                                       name=f"kT{p}") for p in range(2)]
                        qT = [kvq.tile([128, OWN], BF16, tag=f"qT{p}",
                                       name=f"qT{p}") for p in range(2)]
                        vS = [kvq.tile([128, HPS, HD + 1], BF16,
                                       tag=f"vS{tb}", name=f"vS{tb}")
                              for tb in range(SB)]

                        wkq = []
                        for p in range(2):
                            fcol = hs * 256 + p * 128
                            wk_t = wst.tile([128, 8, 128], BF16,
                                            tag=f"wk{p}", name=f"wk{p}")
                            nc.sync.dma_start(
                                wk_t[:],
                                WK[:, fcol:fcol + 128].rearrange(
                                    "(i p2) f -> p2 i f", p2=128))
                            wq_t = wst.tile([128, 8, 128], BF16,
                                            tag=f"wq{p}", name=f"wq{p}")
                            nc.sync.dma_start(
                                wq_t[:],
                                WQ[:, fcol:fcol + 128].rearrange(
                                    "(i p2) f -> p2 i f", p2=128))
                            wkq.append((wk_t, wq_t))
                        wv_t = wst.tile([128, 8, 256], BF16, tag="wv",
                                        name="wv")
                        nc.sync.dma_start(
                            wv_t[:],
                            WV[:, hs * 256:(hs + 1) * 256].rearrange(
                                "(i p2) f -> p2 i f", p2=128))

                        for p in range(2):
                            wk_t, wq_t = wkq[p]
                            bcol = hs * 2 + p
                            for tg in range(4):
                                ps = psKV.tile([128, 512], F32, tag="pk")
                                for db in range(8):
                                    nc.tensor.matmul(
                                        ps[:], wk_t[:, db, :], hT[db][tg][:],
                                        start=(db == 0), stop=(db == 7))
                                nc.vector.tensor_scalar_add(
                                    out=kT[p][:, tg * 512:(tg + 1) * 512],
                                    in0=ps[:],
                                    scalar1=bk_t[:, bcol:bcol + 1])
                            for tg in range(2):
                                ps = psKV.tile([128, 512], F32, tag="pk")
                                for db in range(8):
                                    nc.tensor.matmul(
                                        ps[:], wq_t[:, db, :],
                                        hT[db][2 + tg][:],
                                        start=(db == 0), stop=(db == 7))
                                nc.vector.tensor_scalar_add(
                                    out=qT[p][:, tg * 512:(tg + 1) * 512],
                                    in0=ps[:],
                                    scalar1=bq_t[:, bcol:bcol + 1])

                        # V bias broadcast tile [128, 256] for this head set
                        psb = psKV.tile([128, 512], F32, tag="pk")
                        nc.tensor.matmul(
                            psb[:, 0:256], ones_r[:],
                            bv_t[0:1, hs * 256:(hs + 1) * 256],
                            start=True, stop=True)
                        bvb = att.tile([128, HPS, HD], F32, tag="bvb")
                        nc.scalar.copy(
                            bvb[:],
                            psb[:, 0:256].rearrange("p (h d) -> p h d", d=HD))

                        for tb in range(SB):
                            ps = psKV.tile([128, 512], F32, tag="pk")
                            for db in range(8):
                                nc.tensor.matmul(
                                    ps[:, 0:256],
                                    hT[db][tb // 4][:, (tb % 4) * 128:
                                                    (tb % 4 + 1) * 128],
                                    wv_t[:, db, :],
                                    start=(db == 0), stop=(db == 7))
                            nc.vector.tensor_tensor(
                                out=vS[tb][:, :, 0:HD],
                                in0=ps[:, 0:256].rearrange(
                                    "p (h d) -> p h d", d=HD),
                                in1=bvb[:], op=ALU.add)
                            nc.gpsimd.tensor_copy(vS[tb][:, :, HD:HD + 1],
                                                  ones_c3[:])

                        # ---- attention for this head set ----
                        # PV runs as P.T @ V: the exp block [128k, 128q] is
                        # the stationary operand and V [128k, 65] the bf16
                        # moving operand (65 rows/matmul). The ones column
                        # of V accumulates softmax denominators into the
                        # output's col 64, so normalization is a cheap
                        # per-partition scalar multiply; a PE transpose
                        # brings the normalized [q, feat] block back to
                        # feature-major aT for proj.
                        for g in range(NQG):
                            kl = _klist(g)
                            nquads = g + 1
                            for h in range(HPS):
                                p, sub = h // 2, h % 2
                                # one bank: q-sub accumulators at cols
                                # 0:65 / 128:193, transposed-normalized
                                # output at cols 256:512
                                pab = psA.tile([128, 2 * QG], F32, tag="pab")
                                pq = [pab[:, 0:HD + 1],
                                      pab[:, 128:128 + HD + 1]]
                                for qd in range(nquads):
                                    blocks = kl[4 * qd:4 * qd + 4]
                                    pss = psS.tile([128, 1024], F32,
                                                   tag="ps")
                                    for u in range(4):
                                        kb = blocks[u]
                                        nc.tensor.matmul(
                                            pss[:, u * QG:(u + 1) * QG],
                                            kT[p][sub * 64:(sub + 1) * 64,
                                                  kb * 128:(kb + 1) * 128],
                                            qT[p][sub * 64:(sub + 1) * 64,
                                                  g * QG:(g + 1) * QG],
                                            start=True, stop=True)
                                    wide = att.tile([128, 1024], BF16,
                                                    tag="wide", name="wide")
                                    nc.scalar.activation(wide[:], pss[:],
                                                         AF.Exp)
                                    if qd == g // 2:
                                        sl = wide[:, (g % 2) * 512:
                                                  (g % 2) * 512 + 512]
                                        nc.vector.tensor_mul(
                                            sl, sl, mskE[:, 0, :])
                                    if qd == g:
                                        sl = wide[:, 512:1024]
                                        nc.vector.tensor_mul(
                                            sl, sl, mskE[:, 1, :])
                                    if not am_zero:
                                        for u in range(4):
                                            kb = blocks[u]
                                            sl = wide[:, u * QG:(u + 1) * QG]
                                            nc.vector.tensor_scalar_mul(
                                                out=sl, in0=sl,
                                                scalar1=eam[:, kb:kb + 1])
                                    # one accumulation group for the whole
                                    # bank: start clears the bank-wide
                                    # has_written bits, so only the very
                                    # first matmul may carry it
                                    for u in range(4):
                                        kb = blocks[u]
                                        for qs in range(2):
                                            nc.tensor.matmul(
                                                pq[qs],
                                                wide[:, u * QG + qs * 128:
                                                     u * QG + qs * 128
                                                     + 128],
                                                vS[kb][:, h, :],
                                                start=(qd == 0 and u == 0
                                                       and qs == 0),
                                                stop=(qd == nquads - 1
                                                      and u == 3
                                                      and qs == 1),
                                                skip_group_check=True)
                                ap_idx = 2 * hs + p
                                for qs in range(2):
                                    rec = att.tile([128, 1], F32,
                                                   tag="rec")
                                    nc.vector.reciprocal(
                                        rec[:], pq[qs][:, HD:HD + 1])
                                    anrm = att.tile([128, HD], BF16,
                                                    tag="anrm")
                                    nc.vector.tensor_scalar_mul(
                                        out=anrm[:], in0=pq[qs][:, 0:HD],
                                        scalar1=rec[:])
                                    nc.tensor.transpose(
                                        pab[0:HD, QG + qs * 64:
                                            QG + (qs + 1) * 64].bitcast(
                                                BF16),
                                        anrm[:], ident_b[:])
                                dst = aT[ap_idx][sub * 64:(sub + 1) * 64,
                                                 g * QG:(g + 1) * QG]
                                src = pab[0:HD, QG:QG + 128].bitcast(BF16)
                                if h % 2 == 0:
                                    nc.vector.tensor_copy(dst, src)
                                else:
                                    nc.scalar.copy(dst, src)

            if PL < 4:
                return
            # ---- proj + residual -> x2 ; prefetch WF/BF/BF2 ----
            psT2 = top.enter_context(
                tc.tile_pool(name="psT2", bufs=2, space="PSUM"))
            psF = top.enter_context(
                tc.tile_pool(name="psF", bufs=2, space="PSUM"))
            x2p = top.enter_context(tc.tile_pool(name="x2p", bufs=1,
                                                 side="right"))
            wfp = top.enter_context(tc.tile_pool(name="wfp", bufs=1,
                                                 side="right"))

            # proj weights + residual inputs issue FIRST; the long WF
            # prefetch queues behind them on the SP queue
            wstp = attn_stack.enter_context(tc.tile_pool(name="wstP",
                                                         bufs=1))
            wpt = []
            for fg in range(2):
                w_t = wstp.tile([128, 8, 512], BF16, tag=f"wp{fg}",
                                name=f"wp{fg}")
                nc.sync.dma_start(
                    w_t[:],
                    WP[:, fg * 512:(fg + 1) * 512].rearrange(
                        "(i p2) f -> p2 i f", p2=128))
                wpt.append(w_t)
            x2 = [x2p.tile([128, 4, D], F32, tag=f"x2{i}", name=f"x2{i}")
                  for i in range(2)]
            for i in range(2):
                nc.sync.dma_start(
                    x2[i][:],
                    XQ[i * 512:(i + 1) * 512, :].rearrange(
                        "(i2 p) d -> p i2 d", p=128))

            wf_t = [wfp.tile([128, DFF], BF16, tag=f"wf{db}",
                             name=f"wf{db}") for db in range(8)]
            for db in range(8):
                nc.sync.dma_start(wf_t[db][:],
                                  WF[db * 128:(db + 1) * 128, :])
            bf_t = wfp.tile([128, 32], F32, tag="bf", name="bf")
            nc.sync.dma_start(bf_t[:], BF[:, :])
            bf2_t = wfp.tile([1, D], F32R, tag="bf2", name="bf2")
            nc.sync.dma_start(bf2_t[:], BF2[:, :].bitcast(F32R))

            def x2v(tb):
                return x2[tb // 4][:, tb % 4, :]

            with tc.tile_pool(name="psP", bufs=2, space="PSUM") as psP:
                for tb in range(OB):
                    for fg in range(2):
                        ps = psP.tile([128, 512], F32, tag="pp")
                        for ab in range(8):
                            nc.tensor.matmul(
                                ps[:], aT[ab][:, tb * 128:(tb + 1) * 128],
                                wpt[fg][:, ab, :], start=(ab == 0),
                                stop=(ab == 7))
                        dst = x2v(tb)[:, fg * 512:(fg + 1) * 512]
                        nc.vector.tensor_tensor(out=dst, in0=dst, in1=ps[:],
                                                op=ALU.add)

        if PL < 5:
            return
        # ---- LN2 + transpose -> h2T ; then MLP ----
        with contextlib.ExitStack() as mlp_stack:
            ht2 = mlp_stack.enter_context(tc.tile_pool(name="ht2", bufs=1))
            h2T = [[ht2.tile([128, 512], BF16, tag=f"h2T{db}_{tg}",
                             name=f"h2T{db}_{tg}") for tg in range(2)]
                   for db in range(8)]
            with tc.tile_pool(name="xln2", bufs=6) as xlnp:
                for tg in range(2):
                    xls = []
                    for i in range(4):
                        tb = tg * 4 + i
                        rinv, nb = _ln_stats(nc, stat, x2v(tb), eps_t)
                        x_ln = xlnp.tile([128, D], BF16, tag="xln")
                        nc.scalar.activation(out=x_ln[:], in_=x2v(tb),
                                             func=AF.Identity,
                                             bias=nb[:], scale=rinv[:])
                        xls.append(x_ln)
                    for db in range(8):
                        pt = psT2.tile([128, 512], BF16, tag="tp")
                        for i in range(4):
                            nc.tensor.transpose(
                                pt[:, i * 128:(i + 1) * 128],
                                xls[i][:, db * 128:(db + 1) * 128],
                                ident_b[:])
                        if db % 2 == 0:
                            nc.vector.tensor_copy(h2T[db][tg][:], pt[:])
                        else:
                            nc.scalar.copy(h2T[db][tg][:], pt[:])

            if PL < 6:
                return
            with contextlib.ExitStack() as mlp2:
                gtp = mlp2.enter_context(tc.tile_pool(name="gtp", bufs=1))
                wst6 = mlp2.enter_context(tc.tile_pool(name="wstF6", bufs=4))
                outp = mlp2.enter_context(tc.tile_pool(name="outp", bufs=3))
                psO = None
                for tg in range(2):
                    gt = [gtp.tile([128, 512], BF16, tag=f"gt{j}",
                                   name=f"gt{j}") for j in range(32)]
                    for j in range(32):
                        ps = psF.tile([128, 512], F32, tag="pf")
                        for db in range(8):
                            nc.tensor.matmul(
                                ps[:], wf_t[db][:, j * 128:(j + 1) * 128],
                                h2T[db][tg][:],
                                start=(db == 0), stop=(db == 7))
                        nc.scalar.activation(gt[j][:], ps[:],
                                             AF.Gelu_apprx_tanh,
                                             bias=bf_t[:, j:j + 1], scale=1.0)
                    if psO is None:
                        psO = mlp2.enter_context(
                            tc.tile_pool(name="psO", bufs=1, space="PSUM"))
                    for fg in range(2):
                        last = (tg == 1 and fg == 1)
                        pso = [psO.tile([128, 512], F32, tag=f"po{tb}",
                                        name=f"po{tb}") for tb in range(4)]
                        w8s = []
                        for jj in range(4):
                            w8 = wst6.tile([128, 8, 512], BF16, tag="wf2",
                                           name="wf2")
                            nc.sync.dma_start(
                                w8[:],
                                WF2[jj * 1024:(jj + 1) * 1024,
                                    fg * 512:(fg + 1) * 512].rearrange(
                                        "(i p2) f -> p2 i f", p2=128))
                            w8s.append(w8)
                            if last:
                                continue
                            for jr in range(8):
                                j = jj * 8 + jr
                                for tb in range(4):
                                    nc.tensor.matmul(
                                        pso[tb][:],
                                        gt[j][:, tb * 128:(tb + 1) * 128],
                                        w8[:, jr, :], start=(j == 0),
                                        stop=False)
                        for tb in range(4):
                            if last:
                                # tb-major on the final pass: each output
                                # block drains (bias/add/store) while the
                                # next accumulates, hiding the tail chain
                                for jj in range(4):
                                    for jr in range(8):
                                        j = jj * 8 + jr
                                        nc.tensor.matmul(
                                            pso[tb][:],
                                            gt[j][:, tb * 128:
                                                  (tb + 1) * 128],
                                            w8s[jj][:, jr, :],
                                            start=(j == 0), stop=False)
                            nc.tensor.matmul(
                                pso[tb][:], ones_r[:],
                                bf2_t[0:1, fg * 512:(fg + 1) * 512],
                                start=False, stop=True)
                            gtb = tg * 4 + tb
                            o_t = outp.tile([128, 512], F32, tag="ot")
                            nc.vector.tensor_add(
                                o_t[:], pso[tb][:],
                                x2v(gtb)[:, fg * 512:(fg + 1) * 512])
                            nc.scalar.dma_start(
                                OUT[gtb * 128:(gtb + 1) * 128,
                                    fg * 512:(fg + 1) * 512], o_t[:])


_NC_CACHE = {}


def _get_nc(am_zero=True):
    key = f"nc{int(am_zero)}"
    if key not in _NC_CACHE:
        _NC_CACHE[key] = build_nc(am_zero)
    return _NC_CACHE[key]


def _perm_for(f):
    other = [2 * j + (1 - f) for j in range(8)]
    own = [2 * j + f for j in range(8)]
    blocks = other + own
    return np.concatenate([np.arange(b * 128, (b + 1) * 128) for b in blocks])


def make_in_maps(hidden_states, attention_mask, ln1_g, ln1_b, W_attn, b_attn,
                 W_proj, b_proj, ln2_g, ln2_b, W_fc, b_fc, W_fc2, b_fc2):
    f32 = lambda a: np.asarray(a, dtype=np.float32)
    bf16 = lambda a: np.ascontiguousarray(a.astype(ml_dtypes.bfloat16))
    hidden_states = f32(hidden_states)
    attention_mask = f32(attention_mask)
    ln1_g, ln1_b = f32(ln1_g), f32(ln1_b)
    ln2_g, ln2_b = f32(ln2_g), f32(ln2_b)
    W_attn, b_attn = f32(W_attn), f32(b_attn)
    W_proj, b_proj = f32(W_proj), f32(b_proj)
    W_fc, b_fc = f32(W_fc), f32(b_fc)
    W_fc2, b_fc2 = f32(W_fc2), f32(b_fc2)

    # Fold LN affines into the consuming matmuls (exact algebra, fp64 on host).
    Wa_eff = (ln1_g.astype(np.float64)[:, None] * W_attn).astype(np.float32)
    ba_eff = (b_attn.astype(np.float64)
              + ln1_b.astype(np.float64) @ W_attn).astype(np.float32)
    scale = 1.0 / np.sqrt(np.float32(HD))
    WQn = (Wa_eff[:, 0:D] * scale).astype(np.float32)
    BQn = (ba_eff[0:D] * scale).astype(np.float32)
    WKn, BKn = Wa_eff[:, D:2 * D].copy(), ba_eff[D:2 * D].copy()
    WVn, BVn = Wa_eff[:, 2 * D:3 * D].copy(), ba_eff[2 * D:3 * D].copy()
    Wf_eff = (ln2_g.astype(np.float64)[:, None] * W_fc).astype(np.float32)
    bf_eff = (b_fc.astype(np.float64)
              + ln2_b.astype(np.float64) @ W_fc).astype(np.float32)

    shared = {
        "WQ": bf16(WQn),
        "WK": bf16(WKn),
        "WV": bf16(WVn),
        "BQ": np.ascontiguousarray(BQn.reshape(8, 128).T),
        "BK": np.ascontiguousarray(BKn.reshape(8, 128).T),
        "BV": np.ascontiguousarray(BVn[None, :]),
        "WP": bf16(W_proj),
        "WF": bf16(Wf_eff),
        "BF": np.ascontiguousarray(bf_eff.reshape(32, 128).T),
        "WF2": bf16(W_fc2),
        "BF2": np.ascontiguousarray(b_fc2[None, :]),
    }

    am_zero = bool(np.all(attention_mask == 0))
    in_maps, perms = [], []
    for c in range(N_CORES):
        b, f = c >> 1, c & 1
        perm = _perm_for(f)
        perms.append(perm)
        x_ctx = np.ascontiguousarray(hidden_states[b][perm])
        xq = np.ascontiguousarray(hidden_states[b][perm[OWN:]]
                                  + b_proj[None, :])
        gk = perm
        gq = perm[OWN:]
        live = (gk[:, None] <= gq[None, :]).astype(np.float32)
        # 0/1 exp-masks: [:, u*QG:(u+1)*QG] is k-block (base+u) vs q-group 0
        # pair 0: other-parity blocks (0, 1); pair 1: own blocks (8, 9).
        # The relative pattern is g-independent.
        msk = np.zeros((2, 128, 512), np.float32)
        for u, j in enumerate([0, 1]):
            msk[0, :, u * QG:(u + 1) * QG] = live[
                j * 128:(j + 1) * 128, 0:QG]
        for u, j in enumerate([8, 9]):
            msk[1, :, u * QG:(u + 1) * QG] = live[
                j * 128:(j + 1) * 128, 0:QG]
        im = {"X": bf16(x_ctx), "XQ": xq, "MSKE": bf16(msk), **shared}
        if not am_zero:
            am = attention_mask[b, 0, 0, :].astype(np.float64)
            eam = np.exp(am[perm]).astype(np.float32)
            im["EAM"] = np.ascontiguousarray(eam.reshape(SB, 128).T)
        in_maps.append(im)
    return in_maps, perms, am_zero


def kernel(hidden_states, attention_mask, ln1_g, ln1_b, W_attn, b_attn,
           W_proj, b_proj, ln2_g, ln2_b, W_fc, b_fc, W_fc2, b_fc2):
    in_maps, perms, am_zero = make_in_maps(
        hidden_states, attention_mask, ln1_g, ln1_b, W_attn, b_attn,
        W_proj, b_proj, ln2_g, ln2_b, W_fc, b_fc, W_fc2, b_fc2)
    nc = _get_nc(am_zero)
    res = run_bass_kernel_spmd(nc, in_maps, core_ids=list(range(N_CORES)))
    out = np.empty((B, S, D), dtype=np.float32)
    for c in range(N_CORES):
        b = c >> 1
        out[b][perms[c][OWN:]] = res.results[c]["OUT"]
    return out
